# revision 1
# baseline (speedup 1.0000x reference)
"""Deformable head attention on 8 Trainium2 NeuronCores (Bass/Tile).

Sharding: core c -> batch b = c//4, heads (2*(c%4), 2*(c%4)+1).
Each core samples/attends its two heads and emits a partial (y @ Wm [+bm])
projection [C, HW]; the host sums the 4 partials per batch (unshard).

Slot space per (head, level, h24): slot = hw = k*4096 + w*32 + h4  (16384)
Gather: ap_gather, 2x2 patches (d=4, bf16), groups = (m_loc, h24).
"""
import sys
import numpy as np
import ml_dtypes

sys.path.insert(0, "/opt/trn_rl_repo")

import concourse.bass as bass
import concourse.bacc as bacc
import concourse.tile as tile
from concourse import mybir
from concourse.bass_utils import run_bass_kernel_spmd
from contextlib import ExitStack

F32 = mybir.dt.float32
BF16 = mybir.dt.bfloat16
I16 = mybir.dt.int16
I32 = mybir.dt.int32
AF = mybir.ActivationFunctionType
ALU = mybir.AluOpType
AX = mybir.AxisListType

M, K, L, C = 8, 4, 2, 128
C_v = C // M
B, H, W = 2, 128, 128
HW = H * W
GRIDS = [(64, 64), (128, 128)]
N_CORES = 8
WIN = 1024
NWIN = HW // WIN     # 16

_CACHED = {}


def _build_program():
    P = 128
    nc = bacc.Bacc("TRN2", target_bir_lowering=False, debug=False,
                   num_devices=N_CORES)
    def I(name, shape, dt):
        return nc.dram_tensor(name, shape, dt, kind="ExternalInput").ap()

    zqT_d  = I("zqT",  [C, HW], F32)
    x0T_d  = I("x0T",  [C, 64 * 64], F32)
    x1T_d  = I("x1T",  [C, HW], F32)
    Wcmb_d = I("Wcmb", [C, 48], F32)
    bcmb_d = I("bcmb", [48, 1], F32)
    Wp2_d  = I("Wp2",  [C, 32], F32)
    bp2_d  = I("bp2",  [32, 1], F32)
    F0_d   = I("F0",   [C, C], F32)
    F1_d   = I("F1",   [C, C], F32)
    phix_d = I("phix", [C, 2 * 1024], F32)   # cols l*1024 + sc
    phiy_d = I("phiy", [C, 2 * 1024], F32)
    bm_d   = I("bmv",  [C, 1], F32)
    out_d = nc.dram_tensor("outp", [C, HW], F32, kind="ExternalOutput").ap()

    An_d   = nc.dram_tensor("An_i",   [16, HW], BF16).ap()
    img0_d = nc.dram_tensor("img0_i", [32, 64 * 64 + 64 + 4], BF16).ap()
    img1_d = nc.dram_tensor("img1_i", [32, HW + 128 + 4], BF16).ap()
    part_d = nc.dram_tensor("part_i", [C, HW], F32).ap()

    Ra = np.zeros((16, 128), np.float32)
    for p in range(128):
        Ra[(p // 64) * 8 + (p % 8), p] = 1.0
    Es = np.zeros((16, 2), np.float32)
    for r in range(16):
        Es[r, r // 8] = 1.0

    with tile.TileContext(nc) as tc, ExitStack() as ctx:
        const = ctx.enter_context(tc.tile_pool(name="const", bufs=1))
        Ra_t = const.tile([16, 128], BF16)
        nc.sync.dma_start(Ra_t[:], nc.inline_tensor(Ra.astype(ml_dtypes.bfloat16), name="Ra").ap())
        Es_t = const.tile([16, 2], F32)
        nc.sync.dma_start(Es_t[:], nc.inline_tensor(Es, name="Es").ap())
        F_t = []
        for l, fd in enumerate([F0_d, F1_d]):
            ft = const.tile([C, C], BF16, tag=f"F{l}")
            with tc.tile_pool(name="ftmp", bufs=1) as fp:
                tmp = fp.tile([C, C], F32)
                nc.sync.dma_start(tmp[:], fd)
                nc.vector.tensor_copy(ft[:], tmp[:])
            F_t.append(ft)
        bm_t = const.tile([C, 1], F32)
        nc.sync.dma_start(bm_t[:], bm_d)

        # long-lived pipeline outputs allocated early (stack discipline)
        pipe_out = ctx.enter_context(tc.tile_pool(name="pipeo", bufs=1))
        idx_t = [pipe_out.tile([128, 1024], I16, tag=f"idx{l}", name=f"idxt{l}") for l in range(2)]
        w4_t = [pipe_out.tile([128, 4096], BF16, tag=f"w4{l}", name=f"w4t{l}") for l in range(2)]

        # ========== phase A: DL = Wcmb.T @ zqT + b (zqT streamed) ==========
        DLstack = ExitStack()
        DLpool = DLstack.enter_context(tc.tile_pool(name="DL", bufs=1))
        DLd_t = DLpool.tile([32, HW], F32, tag="DLd")
        DLastack = ExitStack()
        DLapool = DLastack.enter_context(tc.tile_pool(name="DLa", bufs=1))
        DLa_t = DLapool.tile([16, HW], F32, tag="DLa")
        with tc.tile_pool(name="phA", bufs=3) as pa, \
             tc.tile_pool(name="phAp", bufs=2, space="PSUM") as pap:
            Wcmb_t = pa.tile([C, 48], F32, tag="wc")
            nc.sync.dma_start(Wcmb_t[:], Wcmb_d)
            bcmb_t = pa.tile([48, 1], F32, tag="bc")
            nc.sync.dma_start(bcmb_t[:], bcmb_d)
            for w in range(HW // 512):
                zw = pa.tile([C, 512], F32, tag="zw")
                nc.sync.dma_start(zw[:], zqT_d[:, w*512:(w+1)*512])
                d_ps = pap.tile([32, 512], F32, tag="dps")
                nc.tensor.matmul(d_ps[:], Wcmb_t[:, :32], zw[:],
                                 start=True, stop=True)
                nc.scalar.activation(DLd_t[:, w*512:(w+1)*512], d_ps[:],
                                     AF.Identity, bias=bcmb_t[:32, :])
                a_ps = pap.tile([16, 512], F32, tag="aps")
                nc.tensor.matmul(a_ps[:], Wcmb_t[:, 32:48], zw[:],
                                 start=True, stop=True)
                nc.scalar.activation(DLa_t[:, w*512:(w+1)*512], a_ps[:],
                                     AF.Identity, bias=bcmb_t[32:48, :])

        # ========== phase B: softmax -> An (bf16) -> DRAM ==========
        with tc.tile_pool(name="phB", bufs=1) as pb, \
             tc.tile_pool(name="phBp", bufs=2, space="PSUM") as pbp:
            for hq4 in range(4):
                QN = HW // 4
                sl = slice(hq4 * QN, (hq4 + 1) * QN)
                ex = pb.tile([32, QN], F32, tag="ex")
                nc.scalar.activation(ex[:16, :], DLa_t[:, sl], AF.Exp)
                rc = pb.tile([2, QN], F32, tag="rc")
                for w in range(QN // 512):
                    s_ps = pbp.tile([2, 512], F32, tag="sps")
                    nc.tensor.matmul(s_ps[:], Es_t[:], ex[:16, w*512:(w+1)*512],
                                     start=True, stop=True)
                    nc.vector.reciprocal(rc[:, w*512:(w+1)*512], s_ps[:])
                nc.sync.dma_start(ex[16:18, :], rc[:])
                rr = pb.tile([32, QN], F32, tag="rr")
                nc.vector.stream_shuffle(
                    rr[:], ex[:], [16 + (i // 8) for i in range(16)] + list(range(16)))
                anb = pb.tile([16, QN], BF16, tag="anb")
                nc.vector.tensor_tensor(anb[:], ex[:16, :], rr[:16, :], ALU.mult)
                nc.sync.dma_start(An_d[:, sl], anb[:])

        DLastack.close()
        # ========== phase C: coordinate pipeline per level ==========
        # packed: partition p=(m_loc,h24,hp), free sc=(k,w,h4b)  [128,1024]
        with tc.tile_pool(name="phC", bufs=1) as pc:
            phix_t = pc.tile([C, 2048], F32, tag="phx")
            nc.sync.dma_start(phix_t[:], phix_d)
            phiy_t = pc.tile([C, 2048], F32, tag="phy")
            nc.sync.dma_start(phiy_t[:], phiy_d)
            for l, (hl, wl) in enumerate(GRIDS):
                dxp = pc.tile([P, 1024], F32, tag="dxp")
                dyp = pc.tile([P, 1024], F32, tag="dyp")
                # dst col order (k, h4b, w): col = k*256 + hb*128 + w
                for mloc in range(2):
                    for k in range(K):
                        row = mloc * 16 + l * 8 + k * 2
                        for h24 in range(4):
                            pd = mloc * 64 + h24 * 16
                            for xy, dst in ((0, dxp), (1, dyp)):
                                v = DLd_t[row+xy:row+xy+1, :].rearrange(
                                    "r (hb hp hq w) -> r hb hp hq w",
                                    hb=2, hp=16, hq=4, w=128)
                                for hbv in range(2):
                                    vs = v[:, hbv:hbv+1, :, h24:h24+1, :]\
                                        .rearrange("r hb hp hq w -> r (hb hp hq) w")
                                    nc.sync.dma_start(
                                        dst[pd:pd+16,
                                            k*256+hbv*128:k*256+(hbv+1)*128],
                                        vs)
                csx = float(wl) / (wl - 1.0)
                csy = float(hl) / (hl - 1.0)
                # read dxp with (k, w, hb) order to land in sc order
                def sc_view(t):
                    return t[:].rearrange("p (k hb w) -> p k w hb",
                                          k=4, hb=2, w=128)
                ix = pc.tile([P, 1024], F32, tag="ix")
                ixv = ix[:].rearrange("p (k w hb) -> p k w hb", k=4, w=128, hb=2)
                nc.vector.tensor_tensor(
                    ixv, phix_t[:, l*1024:(l+1)*1024]
                    .rearrange("p (k w hb) -> p k w hb", k=4, w=128, hb=2),
                    sc_view(dxp), ALU.add)
                nc.scalar.activation(ix[:], ix[:], AF.Copy, scale=csx, bias=-0.5)
                iy = pc.tile([P, 1024], F32, tag="iy")
                iyv = iy[:].rearrange("p (k w hb) -> p k w hb", k=4, w=128, hb=2)
                nc.vector.tensor_tensor(
                    iyv, phiy_t[:, l*1024:(l+1)*1024]
                    .rearrange("p (k w hb) -> p k w hb", k=4, w=128, hb=2),
                    sc_view(dyp), ALU.add)
                nc.scalar.activation(iy[:], iy[:], AF.Copy, scale=csy, bias=-0.5)

                def floor_clamp(src, hi, tg):
                    sh = pc.tile([P, 1024], F32, tag="fcs")
                    nc.scalar.activation(sh[:], src[:], AF.Copy, bias=-0.5)
                    ii = pc.tile([P, 1024], I32, tag="fci")
                    nc.vector.tensor_copy(ii[:], sh[:])
                    ff = pc.tile([P, 1024], F32, tag="fcf")
                    nc.vector.tensor_copy(ff[:], ii[:])
                    c0 = pc.tile([P, 1024], F32, tag="fc0")
                    nc.vector.tensor_scalar_max(c0[:], ff[:], 0.0)
                    cc = pc.tile([P, 1024], F32, tag=tg)
                    nc.vector.tensor_scalar_min(cc[:], c0[:], float(hi))
                    return cc

                cx = floor_clamp(ix, wl - 2, "cx")
                cy = floor_clamp(iy, hl - 2, "cy")

                def tents(i_t, c_t, tg):
                    tt = pc.tile([P, 1024], F32, tag="tt")
                    nc.vector.tensor_tensor(tt[:], i_t[:], c_t[:], ALU.subtract)
                    pair = pc.tile([P, 2048], F32, tag=tg)
                    pv = pair[:].rearrange("p (n f) -> p n f", f=2)
                    ng = pc.tile([P, 1024], F32, tag="tng")
                    nc.vector.tensor_scalar(ng[:], tt[:], -1.0, None, ALU.mult)
                    a0 = pc.tile([P, 1024], F32, tag="ta0")
                    nc.vector.tensor_tensor(a0[:], tt[:], ng[:], ALU.max)
                    nc.scalar.activation(
                        pv[:, :, 0],
                        a0[:].rearrange("p (n o) -> p n o", o=1)[:, :, 0],
                        AF.Relu, scale=-1.0, bias=1.0)
                    a1 = pc.tile([P, 1024], F32, tag="ta1")
                    nc.vector.tensor_scalar(a1[:], tt[:], -1.0, None, ALU.add)
                    ng1 = pc.tile([P, 1024], F32, tag="tng1")
                    nc.vector.tensor_scalar(ng1[:], a1[:], -1.0, None, ALU.mult)
                    nc.vector.tensor_tensor(a1[:], a1[:], ng1[:], ALU.max)
                    nc.scalar.activation(
                        pv[:, :, 1],
                        a1[:].rearrange("p (n o) -> p n o", o=1)[:, :, 0],
                        AF.Relu, scale=-1.0, bias=1.0)
                    return pair

                u_t = tents(ix, cx, "u")
                v_t = tents(iy, cy, "v")

                idxf = pc.tile([P, 1024], F32, tag="idxf")
                nc.scalar.activation(idxf[:], cy[:], AF.Copy, scale=float(wl))
                nc.vector.tensor_tensor(idxf[:], idxf[:], cx[:], ALU.add)
                nc.vector.tensor_copy(idx_t[l][:], idxf[:])

                wt = w4_t[l]
                wv = wt[:].rearrange("p (n a b) -> p n a b", a=2, b=2)
                nc.vector.tensor_tensor(
                    wv,
                    v_t[:].rearrange("p (n a o) -> p n a o", a=2, o=1)
                        .broadcast_to((P, 1024, 2, 2)),
                    u_t[:].rearrange("p (n o b) -> p n o b", o=1, b=2)
                        .broadcast_to((P, 1024, 2, 2)),
                    ALU.mult)
        DLstack.close()

        # ========== phase D: x projections -> imgs -> DRAM ==========
        for l, (hl, wl) in enumerate(GRIDS):
            ncell = hl * wl
            img_d = img0_d if l == 0 else img1_d
            with tc.tile_pool(name="phD", bufs=2) as pd_, \
                 tc.tile_pool(name="phDp", bufs=2, space="PSUM") as pdp:
                Wp2_t = pd_.tile([C, 32], F32, tag="wp")
                nc.sync.dma_start(Wp2_t[:], Wp2_d)
                bp2_t = pd_.tile([32, 1], F32, tag="bp")
                nc.sync.dma_start(bp2_t[:], bp2_d)
                xd = x0T_d if l == 0 else x1T_d
                for w in range(ncell // 512):
                    xw = pd_.tile([C, 512], F32, tag="xw")
                    nc.sync.dma_start(xw[:], xd[:, w*512:(w+1)*512])
                    ip = pdp.tile([32, 512], F32, tag="ip")
                    nc.tensor.matmul(ip[:], Wp2_t[:], xw[:], start=True, stop=True)
                    ib = pd_.tile([32, 512], BF16, tag="ib")
                    nc.scalar.activation(ib[:], ip[:], AF.Identity, bias=bp2_t[:])
                    nc.sync.dma_start(img_d[:, w*512:(w+1)*512], ib[:])
                zb = pd_.tile([32, wl + 4], BF16, tag="zb")
                nc.vector.memset(zb[:], 0.0)
                nc.sync.dma_start(img_d[:, ncell:], zb[:])

        # ========== phase E: per-level sweep ==========
        for l, (hl, wl) in enumerate(GRIDS):
            ncell = hl * wl
            img_d = img0_d if l == 0 else img1_d
            with tc.tile_pool(name="src3", bufs=1) as ps3, \
                 tc.tile_pool(name="swp", bufs=2) as sw, \
                 tc.tile_pool(name="swp1", bufs=1) as sw1, \
                 tc.tile_pool(name="swpp", bufs=2, space="PSUM") as swp:
                s3 = ps3.tile([P, ncell * 4], BF16)
                s3v = s3[:].rearrange("p (n f) -> p n f", f=4)
                for mloc in range(2):
                    for h24 in range(4):
                        p0 = mloc * 64 + h24 * 16
                        for t in range(4):
                            off = (t // 2) * wl + (t % 2)
                            nc.sync.dma_start(
                                s3v[p0:p0+16, :, t],
                                img_d[mloc*16:(mloc+1)*16, off:off+ncell]
                                .rearrange("p (n o) -> p n o", o=1))
                for w in range(NWIN):
                    anw = sw.tile([16, WIN], BF16, tag="anw")
                    nc.sync.dma_start(anw[:], An_d[:, w*WIN:(w+1)*WIN])
                    an_ps = swp.tile([C, WIN], F32, tag="anps")
                    for q in range(WIN // 512):
                        nc.tensor.matmul(an_ps[:, q*512:(q+1)*512], Ra_t[:],
                                         anw[:, q*512:(q+1)*512],
                                         start=True, stop=True)
                    g_t = sw.tile([P, WIN * 4], BF16, tag="g")
                    nc.gpsimd.ap_gather(
                        g_t[:], s3[:], idx_t[l][:, w*(WIN//16):(w+1)*(WIN//16)],
                        channels=P, num_elems=ncell, d=4, num_idxs=WIN)
                    gv = g_t[:].rearrange("p (n hp f) -> p n hp f", hp=16, f=4)
                    for hp in range(16):
                        wsh = sw.tile([P, (WIN // 16) * 4], BF16, tag="wsh")
                        nc.vector.stream_shuffle(
                            wsh[:], w4_t[l][:, w*(WIN//16)*4:(w+1)*(WIN//16)*4],
                            [(i // 16) * 16 + hp for i in range(32)])
                        nc.vector.tensor_tensor(
                            gv[:, :, hp, :], gv[:, :, hp, :],
                            wsh[:].rearrange("p (n f) -> p n f", f=4), ALU.mult)
                    v1 = sw1.tile([P, WIN], F32, tag="v1")
                    nc.vector.tensor_reduce(
                        v1[:], g_t[:].rearrange("p (n f) -> p n f", f=4),
                        axis=AX.X, op=ALU.add)
                    v2 = sw1.tile([P, WIN], BF16, tag="v2")
                    nc.vector.tensor_tensor(v2[:], v1[:], an_ps[:], ALU.mult)
                    o_ps = swp.tile([C, WIN], F32, tag="ops")
                    for q in range(WIN // 512):
                        nc.tensor.matmul(o_ps[:, q*512:(q+1)*512], F_t[l][:],
                                         v2[:, q*512:(q+1)*512],
                                         start=True, stop=True)
                    ow = sw1.tile([C, WIN], F32, tag="ow")
                    if l == 0:
                        nc.scalar.activation(ow[:], o_ps[:], AF.Identity, bias=bm_t[:])
                        nc.sync.dma_start(part_d[:, w*WIN:(w+1)*WIN], ow[:])
                    else:
                        pw = sw.tile([C, WIN], F32, tag="pw")
                        nc.sync.dma_start(pw[:], part_d[:, w*WIN:(w+1)*WIN])
                        nc.vector.tensor_tensor(ow[:], o_ps[:], pw[:], ALU.add)
                        nc.sync.dma_start(out_d[:, w*WIN:(w+1)*WIN], ow[:])

    nc.compile()
    return nc


def _host_prep(z_q, x0, x1, p_q, Wq, bq, Wd, bd, Wa, ba, Wp, bp, Wm, bm):
    f32 = np.float32
    Wqd_r = (Wq @ Wd).astype(f32).reshape(C, M, L, K, 2)
    bqd_r = (bq @ Wd + bd).astype(f32).reshape(M, L, K, 2)
    Wqa_r = (Wq @ Wa).astype(f32).reshape(C, M, L * K)
    bqa_r = (bq @ Wa + ba).astype(f32).reshape(M, L * K)
    Wp_r = Wp.reshape(C, M, C_v)
    bp_r = bp.reshape(M, C_v)

    # packed coordinate helpers
    pml = np.arange(128) // 64
    ph24 = (np.arange(128) // 16) % 4
    php = np.arange(128) % 16
    kk = np.arange(1024) // 256
    ww = (np.arange(1024) // 2) % 128
    hb = np.arange(1024) % 2
    hq = (hb[None, :] * 16 + php[:, None]) * 4 + ph24[:, None]      # [128,1024]
    wq = np.broadcast_to(ww[None, :], (128, 1024))

    maps = []
    for c in range(N_CORES):
        b = c // 4
        m0 = 2 * (c % 4)
        Wc = np.zeros((C, 48), f32)
        bc = np.zeros((48, 1), f32)
        for ml in range(2):
            m = m0 + ml
            Wc[:, ml*16:(ml+1)*16] = Wqd_r[:, m].reshape(C, 16)
            bc[ml*16:(ml+1)*16, 0] = bqd_r[m].reshape(16)
            Wc[:, 32+ml*8:32+(ml+1)*8] = Wqa_r[:, m]
            bc[32+ml*8:32+(ml+1)*8, 0] = bqa_r[m]
        Wp2 = np.concatenate([Wp_r[:, m0], Wp_r[:, m0+1]], axis=1).astype(f32)
        bp2 = np.concatenate([bp_r[m0], bp_r[m0+1]])[:, None].astype(f32)
        Fs = []
        for l in range(2):
            Fl = np.zeros((C, C), f32)
            for p in range(128):
                ml, h24, j, s = p // 64, (p // 16) % 4, (p % 16) // 8, p % 8
                d2 = h24 * 4 + l * 2 + j
                Fl[p] = Wm[(m0 + ml) * C_v + d2]
            Fs.append(Fl)
        phix = np.zeros((C, 2048), f32)
        phiy = np.zeros((C, 2048), f32)
        par = (m0 + pml) % 2
        for l, (hl, wl) in enumerate(GRIDS):
            pq = p_q[par[:, None], hq, wq]
            phix[:, l*1024:(l+1)*1024] = pq[..., 0] * (wl - 1.0)
            phiy[:, l*1024:(l+1)*1024] = pq[..., 1] * (hl - 1.0)
        lead = (c % 4) == 0
        maps.append(dict(
            zqT=np.ascontiguousarray(z_q[b].reshape(HW, C).T),
            x0T=np.ascontiguousarray(x0[b].reshape(-1, C).T),
            x1T=np.ascontiguousarray(x1[b].reshape(-1, C).T),
            Wcmb=Wc, bcmb=bc, Wp2=Wp2, bp2=bp2,
            F0=Fs[0], F1=Fs[1], phix=phix, phiy=phiy,
            bmv=(bm[:, None].astype(f32) if lead else np.zeros((C, 1), f32)),
        ))
    return maps


def _install_err_capture():
    import traceback, subprocess
    from concourse import bass2jax as b2j
    orig = b2j.neuronx_cc_hook
    def wrapped(*a, **k):
        try:
            return orig(*a, **k)
        except BaseException as e:
            with open("/tmp/ncc_hook_err.txt", "w") as f:
                f.write(traceback.format_exc())
                ee = e
                while ee is not None:
                    if isinstance(ee, subprocess.CalledProcessError):
                        so = ee.stdout if isinstance(ee.stdout, str) else (ee.stdout or b"").decode(errors="replace")
                        f.write("\n==== STDOUT-tail ====\n" + so[-4000:])
                    ee = ee.__cause__ or ee.__context__
            raise
    b2j.neuronx_cc_hook = wrapped
    import libneuronxla
    libneuronxla.neuronx_cc = wrapped


def kernel(**inputs):
    _install_err_capture()
    maps = _host_prep(**{k: np.asarray(v) for k, v in inputs.items()})
    if "nc" not in _CACHED:
        _CACHED["nc"] = _build_program()
    res = run_bass_kernel_spmd(_CACHED["nc"], maps, list(range(N_CORES)))
    out = np.zeros((B, H, W, C), np.float32)
    for c in range(N_CORES):
        out[c // 4] += res.results[c]["outp"].T.reshape(H, W, C)
    return out

